# revision 34
# baseline (speedup 1.0000x reference)
"""Causal self-attention with RoPE on 8 Trainium2 NeuronCores.

Sharding: tensor-parallel over heads (4 groups of 4 heads) x data-parallel
over batch (2), one (batch, head-group) pair per core. Each core computes
its heads' QKV projection, RoPE, causal attention, and a row-slice of the
output projection; the host sums the 4 partial projections per batch.

All matmul operands are bf16 (fp32 PSUM accumulation). Q^T/K^T are
computed directly with the weight chunk stationary and x^T moving (no PE
transposes); RoPE is applied in the transposed layout, exploiting that a
fixed permutation of the head dim cancels in q.k (rotate-half instead of
interleave).

The schedule is a flat unit-interleave per round g: attention chunk-pairs
of group g (the dependency spine, with scores+exp of head h+1 interleaved
against rowsum+PV of head h), the QKV projection chains of group g+1, and
the output-projection units of group g-1, merged evenly so the PE queue
always has ready work while ACT runs exp. Score chunks are computed in
pairs into 2-bank PSUM tiles so one ACTIVATE covers 1024 columns,
halving the 352-cycle fixed cost per exp. Softmax normalization uses
reciprocal_approx_fast on the [1,512] row sums.

Hardcoded problem shape: x (2,2048,2048), Wqkv (2048,6144), Wproj
(2048,2048), cos/sin (2048,64), 16 heads, head_dim 128.
"""

import sys

sys.path.insert(0, "/opt/trn_rl_repo")

import ml_dtypes
import numpy as np

import concourse.bass as bass
import concourse.tile as tile
from concourse import bacc, mybir
from concourse.bass_utils import run_bass_kernel_spmd

B, T, D, H = 2, 2048, 2048, 16
HD, HALF = 128, 64
TPC = 4          # heads per core
NK = D // 128    # 16 contraction chunks for the projections
NG = T // 512    # 4 q/t-groups
NT = T // 128    # 16 key tiles
SCALE = float(1.0 / np.sqrt(HD))
FP32 = mybir.dt.float32
BF16 = mybir.dt.bfloat16
BF16_NP = ml_dtypes.bfloat16
EXP = mybir.ActivationFunctionType.Exp


def build_program():
    nc = bacc.Bacc("TRN2", target_bir_lowering=False, debug=False)

    # x, wp, out are host-pre-tiled so every [128,512] chunk DMA is one
    # contiguous 128KB DRAM block (strided reads run well below peak BW).
    xT = nc.dram_tensor("xT", [NK * NG * 128, 512], BF16, kind="ExternalInput").ap()
    wq = nc.dram_tensor("wq", [D, TPC * HD], BF16, kind="ExternalInput").ap()
    wk = nc.dram_tensor("wk", [D, TPC * HD], BF16, kind="ExternalInput").ap()
    wv = nc.dram_tensor("wv", [D, TPC * HD], BF16, kind="ExternalInput").ap()
    wp = nc.dram_tensor("wp", [16 * 128, 512], BF16, kind="ExternalInput").ap()
    cs = nc.dram_tensor("cs", [128, T], FP32, kind="ExternalInput").ap()
    maskl = nc.dram_tensor("maskl", [128, 128], FP32, kind="ExternalInput").ap()
    ones = nc.dram_tensor("ones", [128, 1], BF16, kind="ExternalInput").ap()
    outT = nc.dram_tensor("outT", [NK * NG * 128, 512], BF16, kind="ExternalOutput").ap()

    with tile.TileContext(nc) as tc:
        _kernel(tc, xT, wq, wk, wv, wp, cs, maskl, ones, outT)
    nc.compile()
    return nc


def _merge(spine, extras):
    """Spread `extras` (order-free units) evenly among `spine` units."""
    if not spine:
        return list(extras)
    out = []
    ns, ne = len(spine), len(extras)
    ei = 0
    for si, s in enumerate(spine):
        out.append(s)
        while ei < ne and (ei + 1) * ns <= (si + 1) * ne:
            out.append(extras[ei])
            ei += 1
    out.extend(extras[ei:])
    return out


def _kernel(tc, xT, wq, wk, wv, wp, cs, maskl, ones, outT):
    nc = tc.nc
    from contextlib import ExitStack

    with ExitStack() as top:
        consts = top.enter_context(tc.tile_pool(name="consts", bufs=1))
        wq_pool = top.enter_context(tc.tile_pool(name="wq", bufs=NK))
        wk_pool = top.enter_context(tc.tile_pool(name="wk", bufs=NK))
        wv_pool = top.enter_context(tc.tile_pool(name="wv", bufs=NK))
        wp_pool = top.enter_context(tc.tile_pool(name="wp", bufs=16))
        x_pool = top.enter_context(tc.tile_pool(name="x", bufs=26))
        qt_pool = top.enter_context(tc.tile_pool(name="qt", bufs=TPC))
        kt_pool = top.enter_context(tc.tile_pool(name="kt", bufs=TPC))
        v_pool = top.enter_context(tc.tile_pool(name="v", bufs=NT))
        o_pool = top.enter_context(tc.tile_pool(name="o", bufs=TPC))
        rope_pool = top.enter_context(tc.tile_pool(name="rope", bufs=1))
        p_pool = top.enter_context(tc.tile_pool(name="p", bufs=11))
        rs_pool = top.enter_context(tc.tile_pool(name="rs", bufs=1))
        rb_pool = top.enter_context(tc.tile_pool(name="rb", bufs=2))
        ob_pool = top.enter_context(tc.tile_pool(name="ob", bufs=3))
        # PSUM banks: s2 2x2 + pp 2 + po 1 + rs 1 = 8
        ps2_pool = top.enter_context(tc.tile_pool(name="ps2", bufs=2, space="PSUM"))
        pp = top.enter_context(tc.tile_pool(name="pp", bufs=2, space="PSUM"))
        ppo = top.enter_context(tc.tile_pool(name="ppo", bufs=1, space="PSUM"))
        ppr = top.enter_context(tc.tile_pool(name="ppr", bufs=1, space="PSUM"))

        l_tile = consts.tile([128, 128], FP32)
        nc.sync.dma_start(out=l_tile, in_=maskl)
        ones_t = consts.tile([128, 1], BF16)
        nc.sync.dma_start(out=ones_t, in_=ones)

        XG = {}

        def xdma(g):
            XG[g] = []
            for k in range(NK):
                xt = x_pool.tile([128, 512], BF16, tag="x")
                r = (k * NG + g) * 128
                nc.scalar.dma_start(out=xt, in_=xT[r : r + 128, :])
                XG[g].append(xt)

        # All DMAs drain through the same 8 HW queues roughly in issue
        # order, so issue in first-use order: wq+x(0) interleaved (the
        # first Q chain is DMA-paced), cos/sin, then wk, wv, x(1), wp.
        WQ = []
        XG[0] = []
        cs_t = None
        for k in range(NK):
            w = wq_pool.tile([128, TPC * HD], BF16, tag="wq")
            nc.sync.dma_start(out=w, in_=wq[k * 128 : (k + 1) * 128, :])
            WQ.append(w)
            xt = x_pool.tile([128, 512], BF16, tag="x")
            nc.scalar.dma_start(out=xt, in_=xT[k * NG * 128 : k * NG * 128 + 128, :])
            XG[0].append(xt)
            if k == 5:
                cs_t = consts.tile([128, T], FP32)   # [cos ; sin] halves
                nc.scalar.dma_start(out=cs_t, in_=cs)
        WK = []
        for k in range(NK):
            w = wk_pool.tile([128, TPC * HD], BF16, tag="wk")
            nc.sync.dma_start(out=w, in_=wk[k * 128 : (k + 1) * 128, :])
            WK.append(w)
        WV = []
        for k in range(NK):
            w = wv_pool.tile([128, TPC * HD], BF16, tag="wv")
            nc.sync.dma_start(out=w, in_=wv[k * 128 : (k + 1) * 128, :])
            WV.append(w)
        xdma(1)
        WP = []  # index hh*4 + m4 -> wp[hh*128:(hh+1)*128, m4*512:(m4+1)*512]
        for hh in range(TPC):
            for m4 in range(4):
                w = wp_pool.tile([128, 512], BF16, tag="wp")
                r = (hh * 4 + m4) * 128
                nc.sync.dma_start(out=w, in_=wp[r : r + 128, :])
                WP.append(w)

        QT = [qt_pool.tile([128, T], BF16, tag="qt", name=f"QT{i}") for i in range(TPC)]
        KT = [kt_pool.tile([128, T], BF16, tag="kt", name=f"KT{i}") for i in range(TPC)]
        V = [v_pool.tile([128, TPC * HD], BF16, tag="v", name=f"V{i}") for i in range(NT)]
        OT = [o_pool.tile([128, T], BF16, tag="o", name=f"OT{i}") for i in range(TPC)]

        # ---------------- unit bodies ----------------

        def qk_chain(g, hh, Wsrc, dstT):
            c0, c1 = g * 512, (g + 1) * 512
            # prefetch the first stationary before this unit's semaphore
            # waits so the load overlaps the previous unit's matmul
            nc.tensor.ldweights(Wsrc[0][:, hh * 128 : (hh + 1) * 128])
            ps = pp.tile([128, 512], FP32, tag="pp")
            for k in range(NK):
                nc.tensor.matmul(
                    ps,
                    Wsrc[k][:, hh * 128 : (hh + 1) * 128],
                    XG[g][k],
                    start=(k == 0),
                    stop=(k == NK - 1),
                )
            # rope: out_lo = q1*c - q2*s ; out_hi = q1*s + q2*c (terms
            # materialized at base 0: SB+SB operands must share a base).
            q1 = ps[0:HALF, :]
            q2 = ps[HALF:128, :]
            cT = cs_t[0:HALF, c0:c1]
            sT = cs_t[HALF:128, c0:c1]
            t1 = rope_pool.tile([HALF, 512], FP32, tag="t1")
            t2 = rope_pool.tile([HALF, 512], FP32, tag="t2")
            t3 = rope_pool.tile([HALF, 512], FP32, tag="t3")
            t4 = rope_pool.tile([HALF, 512], FP32, tag="t4")
            nc.vector.tensor_mul(t1, q1, cT)
            nc.vector.tensor_mul(t2, q2, sT)
            nc.vector.tensor_mul(t3, q1, sT)
            nc.vector.tensor_mul(t4, q2, cT)
            nc.vector.tensor_sub(dstT[hh][0:HALF, c0:c1], t1, t2)
            nc.vector.tensor_add(dstT[hh][HALF:128, c0:c1], t3, t4)

        def v_chain(g, tt):
            nc.tensor.ldweights(XG[g][0][:, tt * 128 : (tt + 1) * 128])
            ps = pp.tile([128, 512], FP32, tag="pp")
            for k in range(NK):
                nc.tensor.matmul(
                    ps,
                    XG[g][k][:, tt * 128 : (tt + 1) * 128],
                    WV[k],
                    start=(k == 0),
                    stop=(k == NK - 1),
                )
            nc.scalar.copy(out=V[4 * g + tt], in_=ps)

        def a2_unit(hh, g, jp, PT2):
            """Two score chunks into one 2-bank PSUM tile + one exp."""
            c0, c1 = g * 512, (g + 1) * 512
            nc.tensor.ldweights(KT[hh][:, 2 * jp * 128 : (2 * jp + 1) * 128])
            ps2 = ps2_pool.tile([128, 1024], FP32, tag="s2")
            pt2 = p_pool.tile([128, 1024], BF16, tag="p")
            ws = []
            for half in range(2):
                kj = 2 * jp + half
                s0 = max(0, kj - 4 * g)
                off = s0 * 128
                w = 512 - off
                base = half * 512
                nc.tensor.matmul(
                    ps2[:, base : base + w],
                    KT[hh][:, kj * 128 : (kj + 1) * 128],
                    QT[hh][:, c0 + off : c1],
                    start=True,
                    stop=True,
                )
                sd = kj - 4 * g
                if 0 <= sd <= 3:
                    dcol = sd * 128 - off
                    nc.vector.tensor_sub(
                        ps2[:, base + dcol : base + dcol + 128],
                        ps2[:, base + dcol : base + dcol + 128],
                        l_tile,
                    )
                ws.append((off, w))
            w1 = ws[1][1]
            nc.scalar.activation(out=pt2[:, : 512 + w1], in_=ps2[:, : 512 + w1],
                                 func=EXP, scale=SCALE)
            PT2.append((pt2, ws))

        def b_unit(st, kj):
            hh, g, nch, PT2 = st["hh"], st["g"], st["nch"], st["PT2"]
            if st["po"] is None:
                st["po"] = ppo.tile([128, 512], FP32, tag="po",
                                    name=f"po{g}_{hh}")
                st["rs"] = ppr.tile([1, 512], FP32, tag="rs",
                                    name=f"rsum{g}_{hh}")
            po, rs = st["po"], st["rs"]
            pt2, ws = PT2[kj // 2]
            off, w = ws[kj % 2]
            base = (kj % 2) * 512
            nc.tensor.matmul(rs[:, off:512], ones_t, pt2[:, base : base + w],
                             start=(kj == 0), stop=(kj == nch - 1))
            nc.tensor.matmul(po[:, off:512],
                             V[kj][:, hh * HD : (hh + 1) * HD],
                             pt2[:, base : base + w],
                             start=(kj == 0), stop=(kj == nch - 1))

        def b_finish(st):
            hh, g = st["hh"], st["g"]
            c0, c1 = g * 512, (g + 1) * 512
            rr = rs_pool.tile([1, 512], FP32, tag="rr")
            nc.vector.tensor_copy(out=rr, in_=st["rs"])
            ri = rs_pool.tile([1, 512], FP32, tag="ri")
            nc.vector.reciprocal_approx_fast(out=ri, in_=rr)
            rrep = rb_pool.tile([128, 512], FP32, tag="rb")
            nc.gpsimd.partition_broadcast(rrep, ri)
            nc.vector.tensor_mul(OT[hh][:, c0:c1], st["po"], rrep)

        def out_unit(g, m):
            c0, c1 = g * 512, (g + 1) * 512
            nc.tensor.ldweights(WP[m // 4][:, (m % 4) * 128 : (m % 4 + 1) * 128])
            ps = pp.tile([128, 512], FP32, tag="pp")
            for hh in range(TPC):
                nc.tensor.matmul(
                    ps,
                    WP[hh * 4 + m // 4][:, (m % 4) * 128 : (m % 4 + 1) * 128],
                    OT[hh][:, c0:c1],
                    start=(hh == 0),
                    stop=(hh == TPC - 1),
                )
            ob = ob_pool.tile([128, 512], BF16, tag="ob")
            nc.scalar.copy(out=ob, in_=ps)
            r = (m * NG + g) * 128
            nc.sync.dma_start(out=outT[r : r + 128, :], in_=ob)

        # ---------------- unit lists ----------------

        def proj_units(g):
            us = []
            for Wsrc, dstT in ((WQ, QT), (WK, KT)):
                for hh in range(TPC):
                    us.append(lambda g=g, hh=hh, Wsrc=Wsrc, dstT=dstT:
                              qk_chain(g, hh, Wsrc, dstT))
            for tt in range(4):
                us.append(lambda g=g, tt=tt: v_chain(g, tt))
            return us

        def attn_spine(g):
            nch = 4 * g + 4
            npair = nch // 2
            units = []
            prev = None
            for hh in range(TPC):
                st = {"hh": hh, "g": g, "nch": nch, "PT2": [], "po": None,
                      "rs": None}
                for jp in range(npair):
                    units.append(lambda hh=hh, g=g, jp=jp, PT2=st["PT2"]:
                                 a2_unit(hh, g, jp, PT2))
                    if prev is not None:
                        def b2(prev=prev, jp=jp):
                            b_unit(prev, 2 * jp)
                            b_unit(prev, 2 * jp + 1)
                        units.append(b2)
                if prev is not None:
                    units.append(lambda prev=prev: b_finish(prev))
                prev = st
            for jp in range(npair):
                def b2(prev=prev, jp=jp):
                    b_unit(prev, 2 * jp)
                    b_unit(prev, 2 * jp + 1)
                units.append(b2)
            units.append(lambda prev=prev: b_finish(prev))
            return units

        def out_units(g):
            return [lambda g=g, m=m: out_unit(g, m) for m in range(NK)]

        # ---------------- schedule ----------------

        for u in proj_units(0):
            u()
        for g in range(NG):
            spine = attn_spine(g)
            extras = []
            if g >= 1:
                extras += out_units(g - 1)   # ready immediately: cover for
            if g + 1 < NG:                   # x(g+1) DMA before proj chains
                extras += proj_units(g + 1)
            for u in _merge(spine, extras):
                u()
            if g + 2 < NG:
                xdma(g + 2)
        for u in out_units(NG - 1):
            u()


_PROGRAM = None


def _get_program():
    global _PROGRAM
    if _PROGRAM is None:
        _PROGRAM = build_program()
    return _PROGRAM


def _make_in_maps(x, cos, sin, Wqkv, Wproj):
    maskl = (np.tril(np.ones((128, 128), np.float32), -1) * 1e30).astype(np.float32)
    ones = np.ones((128, 1), dtype=BF16_NP)
    cosT = np.asarray(cos, np.float32).T   # (64, T)
    sinT = np.asarray(sin, np.float32).T
    cs = np.ascontiguousarray(np.concatenate([cosT, sinT], axis=0))
    in_maps = []
    for c in range(8):
        b, hg = c // 4, c % 4
        h0 = hg * TPC
        # pre-tile x and wp into contiguous [128,512] DMA chunks
        xt = (x[b].T.astype(BF16_NP).reshape(NK, 128, NG, 512)
              .transpose(0, 2, 1, 3).reshape(NK * NG * 128, 512))
        wpt = (Wproj[h0 * HD : (h0 + TPC) * HD, :].astype(BF16_NP)
               .reshape(TPC, 128, 4, 512).transpose(0, 2, 1, 3)
               .reshape(16 * 128, 512))
        in_maps.append({
            "xT": np.ascontiguousarray(xt),
            "wq": np.ascontiguousarray(
                Wqkv[:, h0 * HD : (h0 + TPC) * HD].astype(BF16_NP)),
            "wk": np.ascontiguousarray(
                Wqkv[:, D + h0 * HD : D + (h0 + TPC) * HD].astype(BF16_NP)),
            "wv": np.ascontiguousarray(
                Wqkv[:, 2 * D + h0 * HD : 2 * D + (h0 + TPC) * HD].astype(BF16_NP)),
            "wp": np.ascontiguousarray(wpt),
            "cs": cs,
            "maskl": maskl,
            "ones": ones,
        })
    return in_maps


def _combine(results):
    outs = []
    for b in range(2):
        acc = results[4 * b]["outT"].astype(np.float32)
        for hg in range(1, 4):
            acc = acc + results[4 * b + hg]["outT"].astype(np.float32)
        # un-tile [m, g, 128, 512] -> [D, T]
        full = acc.reshape(NK, NG, 128, 512).transpose(0, 2, 1, 3).reshape(D, T)
        outs.append(full.T)
    return np.ascontiguousarray(np.stack(outs))


def kernel(x, cos, sin, Wqkv, Wproj):
    nc = _get_program()
    in_maps = _make_in_maps(np.asarray(x, np.float32), cos, sin,
                            np.asarray(Wqkv, np.float32), np.asarray(Wproj, np.float32))
    res = run_bass_kernel_spmd(nc, in_maps, list(range(8)))
    return _combine(res.results)


def _install_ntff_shim():
    """Provide the antenv.axon_hooks registry this container lacks, wired to
    the ctypes NTFF hook from trn_agent_boot, so trace=True works."""
    import types

    if "antenv.axon_hooks" in sys.modules:
        return
    hook = None
    try:
        from trn_agent_boot.trn_boot import _ntff_profile_via_ctypes
        hook = _ntff_profile_via_ctypes("/opt/axon/libaxon_pjrt.so")
    except Exception as e:
        print("ntff shim unavailable:", e)
    mod = types.ModuleType("antenv.axon_hooks")
    mod._hook = hook
    mod.get_axon_ntff_profile_hook = lambda: mod._hook
    mod.set_axon_ntff_profile_hook = lambda h: setattr(mod, "_hook", h)
    sys.modules["antenv.axon_hooks"] = mod
    # keep artifacts local; the bucket upload path isn't available here
    import concourse.bass_utils as bu
    bu.upload_artifacts = lambda tmpdir: tmpdir


def kernel_profiled(x, cos, sin, Wqkv, Wproj, trace_cores=None, tmpdir=None):
    nc = _get_program()
    _install_ntff_shim()
    in_maps = _make_in_maps(np.asarray(x, np.float32), cos, sin,
                            np.asarray(Wqkv, np.float32), np.asarray(Wproj, np.float32))
    res = run_bass_kernel_spmd(nc, in_maps, list(range(8)), trace=True,
                               trace_cores=trace_cores, tmpdir=tmpdir)
    return _combine(res.results), res


# revision 35
# speedup vs baseline: 1.0229x; 1.0229x over previous
"""Causal self-attention with RoPE on 8 Trainium2 NeuronCores.

Sharding: tensor-parallel over heads (4 groups of 4 heads) x data-parallel
over batch (2), one (batch, head-group) pair per core. Each core computes
its heads' QKV projection, RoPE, causal attention, and a row-slice of the
output projection; the host sums the 4 partial projections per batch.

All matmul operands are bf16 (fp32 PSUM accumulation). Q^T/K^T are
computed directly with the weight chunk stationary and x^T moving (no PE
transposes); RoPE is applied in the transposed layout, exploiting that a
fixed permutation of the head dim cancels in q.k (rotate-half instead of
interleave).

The schedule is a flat unit-interleave per round g: attention chunk-pairs
of group g (the dependency spine, with scores+exp of head h+1 interleaved
against rowsum+PV of head h), the QKV projection chains of group g+1, and
the output-projection units of group g-1, merged evenly so the PE queue
always has ready work while ACT runs exp. Score chunks are computed in
pairs into 2-bank PSUM tiles so one ACTIVATE covers 1024 columns,
halving the 352-cycle fixed cost per exp. Softmax normalization uses
reciprocal_approx_fast on the [1,512] row sums.

Hardcoded problem shape: x (2,2048,2048), Wqkv (2048,6144), Wproj
(2048,2048), cos/sin (2048,64), 16 heads, head_dim 128.
"""

import sys

sys.path.insert(0, "/opt/trn_rl_repo")

import ml_dtypes
import numpy as np

import concourse.bass as bass
import concourse.tile as tile
from concourse import bacc, mybir
from concourse.bass_utils import run_bass_kernel_spmd

B, T, D, H = 2, 2048, 2048, 16
HD, HALF = 128, 64
TPC = 4          # heads per core
NK = D // 128    # 16 contraction chunks for the projections
NG = T // 512    # 4 q/t-groups
NT = T // 128    # 16 key tiles
SCALE = float(1.0 / np.sqrt(HD))
FP32 = mybir.dt.float32
BF16 = mybir.dt.bfloat16
BF16_NP = ml_dtypes.bfloat16
EXP = mybir.ActivationFunctionType.Exp


def build_program():
    nc = bacc.Bacc("TRN2", target_bir_lowering=False, debug=False)

    # x, wp, out are host-pre-tiled so every [128,512] chunk DMA is one
    # contiguous 128KB DRAM block (strided reads run well below peak BW).
    xT = nc.dram_tensor("xT", [NK * NG * 128, 512], BF16, kind="ExternalInput").ap()
    wq = nc.dram_tensor("wq", [D, TPC * HD], BF16, kind="ExternalInput").ap()
    wk = nc.dram_tensor("wk", [D, TPC * HD], BF16, kind="ExternalInput").ap()
    wv = nc.dram_tensor("wv", [D, TPC * HD], BF16, kind="ExternalInput").ap()
    wp = nc.dram_tensor("wp", [16 * 128, 512], BF16, kind="ExternalInput").ap()
    cs = nc.dram_tensor("cs", [128, T], FP32, kind="ExternalInput").ap()
    maskl = nc.dram_tensor("maskl", [128, 128], FP32, kind="ExternalInput").ap()
    ones = nc.dram_tensor("ones", [128, 1], BF16, kind="ExternalInput").ap()
    outT = nc.dram_tensor("outT", [NK * NG * 128, 512], BF16, kind="ExternalOutput").ap()

    with tile.TileContext(nc) as tc:
        _kernel(tc, xT, wq, wk, wv, wp, cs, maskl, ones, outT)
    nc.compile()
    return nc


def _merge(spine, extras):
    """Spread `extras` (order-free units) evenly among `spine` units."""
    if not spine:
        return list(extras)
    out = []
    ns, ne = len(spine), len(extras)
    ei = 0
    for si, s in enumerate(spine):
        out.append(s)
        while ei < ne and (ei + 1) * ns <= (si + 1) * ne:
            out.append(extras[ei])
            ei += 1
    out.extend(extras[ei:])
    return out


def _kernel(tc, xT, wq, wk, wv, wp, cs, maskl, ones, outT):
    nc = tc.nc
    from contextlib import ExitStack

    with ExitStack() as top:
        consts = top.enter_context(tc.tile_pool(name="consts", bufs=1))
        wq_pool = top.enter_context(tc.tile_pool(name="wq", bufs=NK))
        wk_pool = top.enter_context(tc.tile_pool(name="wk", bufs=NK))
        wv_pool = top.enter_context(tc.tile_pool(name="wv", bufs=NK))
        wp_pool = top.enter_context(tc.tile_pool(name="wp", bufs=16))
        x_pool = top.enter_context(tc.tile_pool(name="x", bufs=26))
        qt_pool = top.enter_context(tc.tile_pool(name="qt", bufs=TPC))
        kt_pool = top.enter_context(tc.tile_pool(name="kt", bufs=TPC))
        v_pool = top.enter_context(tc.tile_pool(name="v", bufs=NT))
        o_pool = top.enter_context(tc.tile_pool(name="o", bufs=TPC))
        rope_pool = top.enter_context(tc.tile_pool(name="rope", bufs=1))
        p_pool = top.enter_context(tc.tile_pool(name="p", bufs=11))
        rs_pool = top.enter_context(tc.tile_pool(name="rs", bufs=1))
        rb_pool = top.enter_context(tc.tile_pool(name="rb", bufs=2))
        ob_pool = top.enter_context(tc.tile_pool(name="ob", bufs=3))
        # PSUM banks: s2 2x2 + pp 2 + po 1 + rs 1 = 8
        ps2_pool = top.enter_context(tc.tile_pool(name="ps2", bufs=2, space="PSUM"))
        pp = top.enter_context(tc.tile_pool(name="pp", bufs=2, space="PSUM"))
        ppo = top.enter_context(tc.tile_pool(name="ppo", bufs=1, space="PSUM"))
        ppr = top.enter_context(tc.tile_pool(name="ppr", bufs=1, space="PSUM"))

        l_tile = consts.tile([128, 128], FP32)
        nc.sync.dma_start(out=l_tile, in_=maskl)
        ones_t = consts.tile([128, 1], BF16)
        nc.sync.dma_start(out=ones_t, in_=ones)

        XG = {}

        def xdma(g):
            XG[g] = []
            for k in range(NK):
                xt = x_pool.tile([128, 512], BF16, tag="x")
                r = (k * NG + g) * 128
                nc.scalar.dma_start(out=xt, in_=xT[r : r + 128, :])
                XG[g].append(xt)

        # All DMAs drain through the same 8 HW queues roughly in issue
        # order, so issue in first-use order: wq+x(0) interleaved (the
        # first Q chain is DMA-paced), cos/sin, then wk, wv, x(1), wp.
        WQ = []
        XG[0] = []
        cs_t = None
        for k in range(NK):
            w = wq_pool.tile([128, TPC * HD], BF16, tag="wq")
            nc.sync.dma_start(out=w, in_=wq[k * 128 : (k + 1) * 128, :])
            WQ.append(w)
            xt = x_pool.tile([128, 512], BF16, tag="x")
            nc.scalar.dma_start(out=xt, in_=xT[k * NG * 128 : k * NG * 128 + 128, :])
            XG[0].append(xt)
            if k == 5:
                cs_t = consts.tile([128, T], FP32)   # [cos ; sin] halves
                nc.scalar.dma_start(out=cs_t, in_=cs)
        WK = []
        for k in range(NK):
            w = wk_pool.tile([128, TPC * HD], BF16, tag="wk")
            nc.sync.dma_start(out=w, in_=wk[k * 128 : (k + 1) * 128, :])
            WK.append(w)
        WV = []
        for k in range(NK):
            w = wv_pool.tile([128, TPC * HD], BF16, tag="wv")
            nc.sync.dma_start(out=w, in_=wv[k * 128 : (k + 1) * 128, :])
            WV.append(w)
        xdma(1)
        WP = []  # index hh*4 + m4 -> wp[hh*128:(hh+1)*128, m4*512:(m4+1)*512]
        for hh in range(TPC):
            for m4 in range(4):
                w = wp_pool.tile([128, 512], BF16, tag="wp")
                r = (hh * 4 + m4) * 128
                nc.sync.dma_start(out=w, in_=wp[r : r + 128, :])
                WP.append(w)

        QT = [qt_pool.tile([128, T], BF16, tag="qt", name=f"QT{i}") for i in range(TPC)]
        KT = [kt_pool.tile([128, T], BF16, tag="kt", name=f"KT{i}") for i in range(TPC)]
        V = [v_pool.tile([128, TPC * HD], BF16, tag="v", name=f"V{i}") for i in range(NT)]
        OT = [o_pool.tile([128, T], BF16, tag="o", name=f"OT{i}") for i in range(TPC)]

        # ---------------- unit bodies ----------------

        def qk_chain(g, hh, Wsrc, dstT):
            c0, c1 = g * 512, (g + 1) * 512
            ps = pp.tile([128, 512], FP32, tag="pp")
            for k in range(NK):
                nc.tensor.matmul(
                    ps,
                    Wsrc[k][:, hh * 128 : (hh + 1) * 128],
                    XG[g][k],
                    start=(k == 0),
                    stop=(k == NK - 1),
                )
            # rope: out_lo = q1*c - q2*s ; out_hi = q1*s + q2*c (terms
            # materialized at base 0: SB+SB operands must share a base).
            q1 = ps[0:HALF, :]
            q2 = ps[HALF:128, :]
            cT = cs_t[0:HALF, c0:c1]
            sT = cs_t[HALF:128, c0:c1]
            t1 = rope_pool.tile([HALF, 512], FP32, tag="t1")
            t2 = rope_pool.tile([HALF, 512], FP32, tag="t2")
            t3 = rope_pool.tile([HALF, 512], FP32, tag="t3")
            t4 = rope_pool.tile([HALF, 512], FP32, tag="t4")
            nc.vector.tensor_mul(t1, q1, cT)
            nc.vector.tensor_mul(t2, q2, sT)
            nc.vector.tensor_mul(t3, q1, sT)
            nc.vector.tensor_mul(t4, q2, cT)
            nc.vector.tensor_sub(dstT[hh][0:HALF, c0:c1], t1, t2)
            nc.vector.tensor_add(dstT[hh][HALF:128, c0:c1], t3, t4)

        def v_chain(g, tt):
            ps = pp.tile([128, 512], FP32, tag="pp")
            for k in range(NK):
                nc.tensor.matmul(
                    ps,
                    XG[g][k][:, tt * 128 : (tt + 1) * 128],
                    WV[k],
                    start=(k == 0),
                    stop=(k == NK - 1),
                )
            nc.scalar.copy(out=V[4 * g + tt], in_=ps)

        def a2_unit(hh, g, jp, PT2):
            """Two score chunks into one 2-bank PSUM tile + one exp."""
            c0, c1 = g * 512, (g + 1) * 512
            ps2 = ps2_pool.tile([128, 1024], FP32, tag="s2")
            pt2 = p_pool.tile([128, 1024], BF16, tag="p")
            ws = []
            for half in range(2):
                kj = 2 * jp + half
                s0 = max(0, kj - 4 * g)
                off = s0 * 128
                w = 512 - off
                base = half * 512
                nc.tensor.matmul(
                    ps2[:, base : base + w],
                    KT[hh][:, kj * 128 : (kj + 1) * 128],
                    QT[hh][:, c0 + off : c1],
                    start=True,
                    stop=True,
                )
                sd = kj - 4 * g
                if 0 <= sd <= 3:
                    dcol = sd * 128 - off
                    nc.vector.tensor_sub(
                        ps2[:, base + dcol : base + dcol + 128],
                        ps2[:, base + dcol : base + dcol + 128],
                        l_tile,
                    )
                ws.append((off, w))
            w1 = ws[1][1]
            nc.scalar.activation(out=pt2[:, : 512 + w1], in_=ps2[:, : 512 + w1],
                                 func=EXP, scale=SCALE)
            PT2.append((pt2, ws))

        def b_unit(st, kj):
            hh, g, nch, PT2 = st["hh"], st["g"], st["nch"], st["PT2"]
            if st["po"] is None:
                st["po"] = ppo.tile([128, 512], FP32, tag="po",
                                    name=f"po{g}_{hh}")
                st["rs"] = ppr.tile([1, 512], FP32, tag="rs",
                                    name=f"rsum{g}_{hh}")
            po, rs = st["po"], st["rs"]
            pt2, ws = PT2[kj // 2]
            off, w = ws[kj % 2]
            base = (kj % 2) * 512
            nc.tensor.matmul(rs[:, off:512], ones_t, pt2[:, base : base + w],
                             start=(kj == 0), stop=(kj == nch - 1))
            nc.tensor.matmul(po[:, off:512],
                             V[kj][:, hh * HD : (hh + 1) * HD],
                             pt2[:, base : base + w],
                             start=(kj == 0), stop=(kj == nch - 1))

        def b_finish(st):
            hh, g = st["hh"], st["g"]
            c0, c1 = g * 512, (g + 1) * 512
            rr = rs_pool.tile([1, 512], FP32, tag="rr")
            nc.vector.tensor_copy(out=rr, in_=st["rs"])
            ri = rs_pool.tile([1, 512], FP32, tag="ri")
            nc.vector.reciprocal_approx_fast(out=ri, in_=rr)
            rrep = rb_pool.tile([128, 512], FP32, tag="rb")
            nc.gpsimd.partition_broadcast(rrep, ri)
            nc.vector.tensor_mul(OT[hh][:, c0:c1], st["po"], rrep)

        def out_unit(g, m):
            c0, c1 = g * 512, (g + 1) * 512
            ps = pp.tile([128, 512], FP32, tag="pp")
            for hh in range(TPC):
                nc.tensor.matmul(
                    ps,
                    WP[hh * 4 + m // 4][:, (m % 4) * 128 : (m % 4 + 1) * 128],
                    OT[hh][:, c0:c1],
                    start=(hh == 0),
                    stop=(hh == TPC - 1),
                )
            ob = ob_pool.tile([128, 512], BF16, tag="ob")
            nc.scalar.copy(out=ob, in_=ps)
            r = (m * NG + g) * 128
            nc.sync.dma_start(out=outT[r : r + 128, :], in_=ob)

        # ---------------- unit lists ----------------

        def proj_units(g):
            us = []
            for Wsrc, dstT in ((WQ, QT), (WK, KT)):
                for hh in range(TPC):
                    us.append(lambda g=g, hh=hh, Wsrc=Wsrc, dstT=dstT:
                              qk_chain(g, hh, Wsrc, dstT))
            for tt in range(4):
                us.append(lambda g=g, tt=tt: v_chain(g, tt))
            return us

        def attn_spine(g):
            nch = 4 * g + 4
            npair = nch // 2
            units = []
            prev = None
            for hh in range(TPC):
                st = {"hh": hh, "g": g, "nch": nch, "PT2": [], "po": None,
                      "rs": None}
                for jp in range(npair):
                    units.append(lambda hh=hh, g=g, jp=jp, PT2=st["PT2"]:
                                 a2_unit(hh, g, jp, PT2))
                    if prev is not None:
                        def b2(prev=prev, jp=jp):
                            b_unit(prev, 2 * jp)
                            b_unit(prev, 2 * jp + 1)
                        units.append(b2)
                if prev is not None:
                    units.append(lambda prev=prev: b_finish(prev))
                prev = st
            for jp in range(npair):
                def b2(prev=prev, jp=jp):
                    b_unit(prev, 2 * jp)
                    b_unit(prev, 2 * jp + 1)
                units.append(b2)
            units.append(lambda prev=prev: b_finish(prev))
            return units

        def out_units(g):
            return [lambda g=g, m=m: out_unit(g, m) for m in range(NK)]

        # ---------------- schedule ----------------

        for u in proj_units(0):
            u()
        for g in range(NG):
            spine = attn_spine(g)
            extras = []
            if g >= 1:
                extras += out_units(g - 1)   # ready immediately: cover for
            if g + 1 < NG:                   # x(g+1) DMA before proj chains
                extras += proj_units(g + 1)
            for u in _merge(spine, extras):
                u()
            if g + 2 < NG:
                xdma(g + 2)
        for u in out_units(NG - 1):
            u()


_PROGRAM = None


def _get_program():
    global _PROGRAM
    if _PROGRAM is None:
        _PROGRAM = build_program()
    return _PROGRAM


def _make_in_maps(x, cos, sin, Wqkv, Wproj):
    maskl = (np.tril(np.ones((128, 128), np.float32), -1) * 1e30).astype(np.float32)
    ones = np.ones((128, 1), dtype=BF16_NP)
    cosT = np.asarray(cos, np.float32).T   # (64, T)
    sinT = np.asarray(sin, np.float32).T
    cs = np.ascontiguousarray(np.concatenate([cosT, sinT], axis=0))
    in_maps = []
    for c in range(8):
        b, hg = c // 4, c % 4
        h0 = hg * TPC
        # pre-tile x and wp into contiguous [128,512] DMA chunks
        xt = (x[b].T.astype(BF16_NP).reshape(NK, 128, NG, 512)
              .transpose(0, 2, 1, 3).reshape(NK * NG * 128, 512))
        wpt = (Wproj[h0 * HD : (h0 + TPC) * HD, :].astype(BF16_NP)
               .reshape(TPC, 128, 4, 512).transpose(0, 2, 1, 3)
               .reshape(16 * 128, 512))
        in_maps.append({
            "xT": np.ascontiguousarray(xt),
            "wq": np.ascontiguousarray(
                Wqkv[:, h0 * HD : (h0 + TPC) * HD].astype(BF16_NP)),
            "wk": np.ascontiguousarray(
                Wqkv[:, D + h0 * HD : D + (h0 + TPC) * HD].astype(BF16_NP)),
            "wv": np.ascontiguousarray(
                Wqkv[:, 2 * D + h0 * HD : 2 * D + (h0 + TPC) * HD].astype(BF16_NP)),
            "wp": np.ascontiguousarray(wpt),
            "cs": cs,
            "maskl": maskl,
            "ones": ones,
        })
    return in_maps


def _combine(results):
    outs = []
    for b in range(2):
        acc = results[4 * b]["outT"].astype(np.float32)
        for hg in range(1, 4):
            acc = acc + results[4 * b + hg]["outT"].astype(np.float32)
        # un-tile [m, g, 128, 512] -> [D, T]
        full = acc.reshape(NK, NG, 128, 512).transpose(0, 2, 1, 3).reshape(D, T)
        outs.append(full.T)
    return np.ascontiguousarray(np.stack(outs))


def kernel(x, cos, sin, Wqkv, Wproj):
    nc = _get_program()
    in_maps = _make_in_maps(np.asarray(x, np.float32), cos, sin,
                            np.asarray(Wqkv, np.float32), np.asarray(Wproj, np.float32))
    res = run_bass_kernel_spmd(nc, in_maps, list(range(8)))
    return _combine(res.results)


def _install_ntff_shim():
    """Provide the antenv.axon_hooks registry this container lacks, wired to
    the ctypes NTFF hook from trn_agent_boot, so trace=True works."""
    import types

    if "antenv.axon_hooks" in sys.modules:
        return
    hook = None
    try:
        from trn_agent_boot.trn_boot import _ntff_profile_via_ctypes
        hook = _ntff_profile_via_ctypes("/opt/axon/libaxon_pjrt.so")
    except Exception as e:
        print("ntff shim unavailable:", e)
    mod = types.ModuleType("antenv.axon_hooks")
    mod._hook = hook
    mod.get_axon_ntff_profile_hook = lambda: mod._hook
    mod.set_axon_ntff_profile_hook = lambda h: setattr(mod, "_hook", h)
    sys.modules["antenv.axon_hooks"] = mod
    # keep artifacts local; the bucket upload path isn't available here
    import concourse.bass_utils as bu
    bu.upload_artifacts = lambda tmpdir: tmpdir


def kernel_profiled(x, cos, sin, Wqkv, Wproj, trace_cores=None, tmpdir=None):
    nc = _get_program()
    _install_ntff_shim()
    in_maps = _make_in_maps(np.asarray(x, np.float32), cos, sin,
                            np.asarray(Wqkv, np.float32), np.asarray(Wproj, np.float32))
    res = run_bass_kernel_spmd(nc, in_maps, list(range(8)), trace=True,
                               trace_cores=trace_cores, tmpdir=tmpdir)
    return _combine(res.results), res
